# revision 77
# baseline (speedup 1.0000x reference)
"""PointPillarsScatter Trainium2 kernel (fp16 pipeline).

Reference op:
  canvas[b*NY*NX + y*NX + x] = voxel_features[p]        (scatter-set, 64 ch)
  out[:, :64]  = canvas -> [B, 64, NY, NX]
  out[:, 64:]  = transpose(map_fm, (0, 3, 2, 1))        (16 ch)

Strategy (8 NeuronCores, SPMD, data-parallel per sharding hint):
  core = batch*2 + y_half  (4 batches x 2 halves of NY=496 -> NYH=248 rows).

  Everything on-device runs in fp16 (correctness gate is rel_err < 2e-2;
  fp16 gives ~4e-4), which halves HBM traffic vs fp32 and lets the PE run
  at 1 column/cycle.  The scatter is a one-hot matmul on the TensorEngine:
    psum[128ch', 512cells] = feat[Hslots, 128ch'].T @ S[Hslots, 512]
  where S[s, n] = (pos[s] == n) is built on the VectorEngine with
  iota + is_equal (fp16 in/out -> 2x DVE mode; the per-partition scalar
  pos operand must be fp32), and ch' packs the 64 channels of TWO
  512-cell tiles (tile j -> psum partitions 0:64, tile j+105 -> 64:128).
  This fuses zero-fill + scatter + transpose into one PE op per 1024
  cells.  With fp16 there is no hi/lo split; the slot height H is the max
  points-per-column rounded up to 16 (typically 96), which trims the
  zero-padded feature-table DMA.

  The canvas DRAM layout is [128, ACELL] fp16 (partition = half*64 + ch),
  so every store is one full-128-partition DMA with 8KB runs; the host
  reassembles the [64, NCELL] canvas from the two halves.

  map_fm is transposed with PE transpose (fp16 identity) in [<=128, 128]
  blocks directly into fp16 PSUM (1 bank per 2 y-blocks), copied
  2 y-blocks at a time on the VectorEngine, and stored 4 y-blocks per
  DMA (3456B contiguous runs), paced across the scatter groups.

  Pipeline shape (measured-best): constants/pos-table/feature groups
  load via HWDGE while the map input rides the GpSimd SWDGE queue -- the
  device spends the first ~10us as a pure, DMA-saturated load phase,
  then runs a dense compute+store phase.  Starting compute before the
  inputs are resident consistently measured SLOWER: early PE data-stalls
  re-throttle the PE HAM clock gate to 1.2GHz and the stream never
  re-warms.  Tried and regressed vs this shape: on-device constants with
  early is_equal (91us), delayed/halved map loads (93us), warmup and
  re-warm dummy-matmul bursts (93-97us), DMA-transpose map loads
  (156us), FBLOOK=6 deep feat prefetch at setup (98us), is_equal
  on GpSimd (259us; 7.9us/op vs 262ns on DVE).  Wins on top: in-loop
  feat loads via SWDGE (sync queue left to stores; 83->79us) and map
  work paced over the first 8 groups (79->77us; 6 overshoots to 81us).
  PSUM->SBUF copies split
  ~6:1 between ScalarE and VectorE balances all three compute engines at
  ~48us, under the ~62us DMA floor for 23.4MB/core.

Host side only computes index tables + shards/casts inputs; all FP math
(scatter + transpose) runs on device.
"""

import sys

for _p in ("/opt/trn_rl_repo",):
    if _p not in sys.path:
        sys.path.insert(0, _p)

import numpy as np

# problem constants (hardcoded per contract)
B, NPTS, C, NY, NX, CM = 4, 48000, 64, 496, 432, 16
NYH = NY // 2            # 248 rows per core
NCORE = 8
NCELL = NYH * NX         # 107136 cells per core
TILE = 512               # cells per channel-block
NT = (NCELL + TILE - 1) // TILE          # 210 tiles (last has 128 cells)
NP = (NT + 1) // 2                       # 105 pairs: tile j with tile j+NP
ACELL = NP * TILE                        # 53760 cells in the A half
BCELL = NCELL - ACELL                    # 53376 cells in the B half
CAP = 128                # point slots per column (fp16: no hi/lo split)
SG = 8                   # pairs per canvas-store group (105 = 13*8 + 1)
NSG = (NP + SG - 1) // SG                # 14 store groups
FBMAX = 12               # max feat columns per group load
FBLOOK = 3               # feat groups prefetched ahead
YB = 8                   # map y rows per transpose block ( YB*CM = 128 )
NYB = NYH // YB          # 31 y-blocks
NMU = (NYB + 1) // 2     # 16 map units (2 y-blocks each; last has 1)
MBG = 4                  # map y-blocks per store DMA (8 stores)
MAP_BY = 8               # scatter groups over which map work is spread
XCH = [(0, 128), (128, 128), (256, 128), (384, 48)]   # x chunks of NX=432

_prog_cache = {}


def _build_program(ncols, chunks, hmax):
    """Build the SPMD Bass program (identical for all 8 cores)."""
    from concourse import bacc, mybir, tile

    f16 = mybir.dt.float16
    f32 = mybir.dt.float32

    nc = bacc.Bacc(trn_type="TRN2", target_bir_lowering=False)

    # slot-major layout: partition s reads one contiguous run per load
    feat_d = nc.dram_tensor("feat", [hmax, ncols * 2 * C], f16,
                            kind="ExternalInput")
    post_d = nc.dram_tensor("post", [hmax, ncols], f32, kind="ExternalInput")
    aux_d = nc.dram_tensor("aux", [128, TILE + 128], f16,
                           kind="ExternalInput")
    map_d = nc.dram_tensor("mapin", [NX, NYH * CM], f16, kind="ExternalInput")
    canv_d = nc.dram_tensor("canv", [128, ACELL], f16, kind="ExternalOutput")
    mapo_d = nc.dram_tensor("mapo", [128, NYB * NX], f16,
                            kind="ExternalOutput")

    colbase = np.concatenate([[0], np.cumsum(chunks)]).astype(np.int64)

    with tile.TileContext(nc) as tc:
        with (
            tc.tile_pool(name="const", bufs=1) as cpool,
            tc.tile_pool(name="fpool", bufs=FBLOOK + 1) as fpool,
            tc.tile_pool(name="spool", bufs=8) as spool,
            tc.tile_pool(name="stg", bufs=4) as stpool,
            tc.tile_pool(name="mstg", bufs=3) as mstpool,
            tc.tile_pool(name="mtin", bufs=1) as mtpool,
            tc.tile_pool(name="pscat", bufs=3, space="PSUM") as pspool,
            tc.tile_pool(name="pmap", bufs=2, space="PSUM") as pmpool,
        ):
            # constants + pos table via DMA
            aux = cpool.tile([128, TILE + 128], f16)
            nc.scalar.dma_start(out=aux[:], in_=aux_d[:])
            iota_f = aux[:, 0:TILE]
            ident = aux[:, TILE:TILE + 128]
            posT = cpool.tile([hmax, ncols], f32)
            nc.scalar.dma_start(out=posT[:], in_=post_d[:])

            fbs = {}

            def load_fb(g, eng=None):
                # in-loop loads ride the idle GpSimd (SWDGE) queue so they
                # never queue behind store triggers waiting on copies
                p0, p1 = g * SG, min((g + 1) * SG, NP)
                c0, c1 = int(colbase[p0]), int(colbase[p1])
                assert c1 - c0 <= FBMAX, (c0, c1)
                fb = fpool.tile([hmax, FBMAX * 2 * C], f16, name="fb")
                (eng or nc.gpsimd).dma_start(
                    out=fb[:, :(c1 - c0) * 2 * C],
                    in_=feat_d[:, c0 * 2 * C:c1 * 2 * C])
                fbs[g] = (fb, c0, c1)

            for g in range(min(FBLOOK, NSG)):
                load_fb(g, eng=nc.sync)

            # map input rides the idle GpSimd (SWDGE) queue
            mts = []
            for xi, (x0, w) in enumerate(XCH):
                mt = mtpool.tile([128, NYH * CM], f16, tag="mt%d" % x0)
                nc.gpsimd.dma_start(out=mt[:w, :], in_=map_d[x0:x0 + w, :])
                mts.append(mt)

            # ---- map transpose machinery ----
            # unit k2 covers y-blocks 2*k2, 2*k2+1 (last unit: 1 block)
            mstate = {"ms": None}

            def emit_map_unit(k2):
                nb = 2 if 2 * k2 + 1 < NYB else 1
                pm = pmpool.tile([128, 2 * NX], f16, name="pm")
                for j in range(nb):
                    kb = 2 * k2 + j
                    for xi, (x0, w) in enumerate(XCH):
                        nc.tensor.transpose(
                            out=pm[:, j * NX + x0:j * NX + x0 + w],
                            in_=mts[xi][:w, kb * 128:(kb + 1) * 128],
                            identity=ident[:w, :w])
                if k2 % 2 == 0:
                    mstate["ms"] = mstpool.tile([128, MBG * NX], f16,
                                                name="ms")
                ms = mstate["ms"]
                off = (k2 % 2) * 2 * NX
                nc.vector.tensor_copy(out=ms[:, off:off + nb * NX],
                                      in_=pm[:, :nb * NX])
                if k2 % 2 == 1 or k2 == NMU - 1:
                    blk0 = (k2 - k2 % 2) * 2
                    wm = (min(blk0 + MBG, NYB) - blk0) * NX
                    nc.sync.dma_start(
                        out=mapo_d[:, blk0 * NX:blk0 * NX + wm],
                        in_=ms[:, :wm])

            # ---- scatter main loop ----
            emitted_units = 0
            ndp = 0     # global double-pair counter (for copy-engine split)
            for g in range(NSG):
                p0, p1 = g * SG, min((g + 1) * SG, NP)
                fb, c0, c1 = fbs[g]
                stg = stpool.tile([128, SG * TILE], f16, name="stg")
                pr = p0
                while pr < p1:
                    npair = min(2, p1 - pr)
                    ps = pspool.tile([128, 2 * TILE], f32, name="ps")
                    for q in range(npair):
                        pcur = pr + q
                        nck = int(chunks[pcur])
                        for k in range(nck):
                            col = int(colbase[pcur]) + k
                            s_t = spool.tile([hmax, TILE], f16, name="s_t")
                            nc.vector.tensor_scalar(
                                out=s_t[:], in0=iota_f[:hmax],
                                scalar1=posT[:, col:col + 1], scalar2=None,
                                op0=mybir.AluOpType.is_equal)
                            nc.tensor.matmul(
                                out=ps[:, q * TILE:(q + 1) * TILE],
                                lhsT=fb[:, (col - c0) * 2 * C:
                                        (col - c0 + 1) * 2 * C],
                                rhs=s_t[:],
                                start=(k == 0), stop=(k == nck - 1))
                    off = (pr - p0) * TILE
                    # most copies on ACT; every 7th on DVE for balance
                    if ndp % 7 == 6:
                        nc.vector.tensor_copy(
                            out=stg[:, off:off + npair * TILE],
                            in_=ps[:, :npair * TILE])
                    else:
                        nc.scalar.copy(
                            out=stg[:, off:off + npair * TILE],
                            in_=ps[:, :npair * TILE])
                    ndp += 1
                    pr += npair
                a0 = p0 * TILE
                wa = (p1 - p0) * TILE
                nc.sync.dma_start(out=canv_d[:, a0:a0 + wa],
                                  in_=stg[:, :wa])
                if g + FBLOOK < NSG:
                    load_fb(g + FBLOOK)
                while (emitted_units < NMU
                       and emitted_units * MAP_BY < (g + 1) * NMU):
                    emit_map_unit(emitted_units)
                    emitted_units += 1
            while emitted_units < NMU:
                emit_map_unit(emitted_units)
                emitted_units += 1

    nc.finalize()
    return nc


def _host_prep(voxel_features, coords, map_fm):
    """Shard points by core, build fp16 feature/pos tables (index work)."""
    vf = np.asarray(voxel_features)
    cd = np.asarray(coords)
    mf = np.asarray(map_fm)
    if mf.ndim == 5:
        mf = np.squeeze(mf, 3)

    b = cd[:, 0].astype(np.int64)
    y = cd[:, 2].astype(np.int64)
    x = cd[:, 3].astype(np.int64)
    valid = (b >= 0) & (b < B) & (y >= 0) & (y < NY) & (x >= 0) & (x < NX)
    b, y, x = b[valid], y[valid], x[valid]
    vfv = np.ascontiguousarray(vf[valid]).astype(np.float16)

    half = (y >= NYH).astype(np.int64)
    core = b * 2 + half
    lcell = (y - half * NYH) * NX + x
    t = lcell // TILE          # 512-cell tile id
    pos = lcell - t * TILE     # position within tile (= matmul column)
    pair = t % NP              # tile j pairs with tile j+NP
    blk = t // NP              # channel block within the pair

    key = core * NP + pair
    order = np.argsort(key, kind="stable")
    ks = key[order]
    counts = np.bincount(ks, minlength=NCORE * NP)
    kmax = counts.reshape(NCORE, NP).max(axis=0)
    # table height: round max points-per-column up to a multiple of 16
    hmax = int(min(CAP, max(16, -(-int(kmax.max()) // 16) * 16)))
    chunks = np.maximum((kmax + hmax - 1) // hmax, 1)
    for g in range(0, NP, SG):
        need = int(chunks[g:g + SG].sum())
        if need > FBMAX:
            raise ValueError("pair group needs %d cols > FBMAX=%d"
                             % (need, FBMAX))
    ncols = int(chunks.sum())
    colbase = np.concatenate([[0], np.cumsum(chunks)]).astype(np.int64)

    starts = np.concatenate([[0], np.cumsum(counts)]).astype(np.int64)
    rank = np.arange(len(ks), dtype=np.int64) - starts[ks]

    co = core[order]
    po = pair[order]
    bo = blk[order]
    colo = colbase[po] + rank // hmax
    slot = rank % hmax

    feat = np.zeros((NCORE, hmax, ncols, 2 * C), np.float16)
    post = np.full((NCORE, hmax, ncols), -1.0, np.float32)
    ccol = bo[:, None] * C + np.arange(C)[None, :]
    feat[co[:, None], slot[:, None], colo[:, None], ccol] = vfv[order]
    post[co, slot, colo] = pos[order].astype(np.float32)

    maps = []
    for core_id in range(NCORE):
        bb, hh = core_id // 2, core_id % 2
        maps.append(np.ascontiguousarray(
            mf[bb, :, hh * NYH:(hh + 1) * NYH, :]).astype(
                np.float16).reshape(NX, NYH * CM))
    return feat, post, maps, ncols, chunks, hmax


def kernel(voxel_features, coords, batch_size=None, map_fm=None,
           trace=False, _return_results=False):
    from concourse.bass_utils import run_bass_kernel_spmd

    feat, post, maps, ncols, chunks, hmax = _host_prep(
        voxel_features, coords, map_fm)

    ckey = (ncols, hmax, tuple(int(c) for c in chunks))
    if ckey not in _prog_cache:
        _prog_cache.clear()
        _prog_cache[ckey] = _build_program(ncols, chunks, hmax)
    nc = _prog_cache[ckey]

    aux = np.concatenate(
        [np.broadcast_to(np.arange(TILE, dtype=np.float16), (128, TILE)),
         np.eye(128, dtype=np.float16)], axis=1)
    aux = np.ascontiguousarray(aux)
    in_maps = [
        {"feat": feat[i].reshape(hmax, -1), "post": post[i],
         "aux": aux, "mapin": maps[i]}
        for i in range(NCORE)
    ]
    res = run_bass_kernel_spmd(nc, in_maps, list(range(NCORE)), trace=trace)

    out = np.empty((B, C + CM, NY, NX), np.float32)
    for core_id in range(NCORE):
        bb, hh = core_id // 2, core_id % 2
        canv = res.results[core_id]["canv"]          # [128, ACELL] f16
        full = np.concatenate(
            [canv[0:C], canv[C:, :BCELL]], axis=1).astype(np.float32)
        out[bb, :C, hh * NYH:(hh + 1) * NYH, :] = full.reshape(C, NYH, NX)
        mo = res.results[core_id]["mapo"]            # [128, NYB*NX] f16
        out[bb, C:, hh * NYH:(hh + 1) * NYH, :] = (
            mo.reshape(YB, CM, NYB, NX).transpose(1, 2, 0, 3)
            .astype(np.float32).reshape(CM, NYH, NX))
    if _return_results:
        return out, res
    return out
